# revision 30
# baseline (speedup 1.0000x reference)
"""DrBC GNN message-passing kernel for 8 Trainium2 NeuronCores.

Strategy (graph/data parallel, per sharding hint):
  - Nodes are sharded by range across the 8 cores after a load-balancing
    permutation (snake assignment of in-degree-sorted nodes to the 800
    (core, tile) buckets), so every 128-node dest tile has an in-degree
    sum of ~1000 and its edge chunks pack near-optimally.
  - The irregular gather h[row] uses GPSIMD dma_gather from a replicated
    DRAM table g = dinv * h (bf16), windowed into 4 source ranges that
    coincide with the AllGather chunks (each <= 32768 rows, the int16
    index limit).  Q7 descriptor generation is the hard serial floor
    (~7.8 ns/row); the "hardware" indirect-DMA path measures slower
    (~11 ns/row) and shares the same descriptor unit, so plain
    dma_gather on queue 0 is the fastest mechanism available.
  - segment_sum runs on the TensorEngine: per 128-edge chunk,
    aggr[f, v] += gath[e, f]^T @ S[e, v], S one-hot scaled by dinv[col],
    streamed from HBM in bf16; PSUM accumulates fp32.  Matmuls are
    emitted grouped by dest tile so PSUM accumulation runs stay
    contiguous.
  - GRU runs feature-major ([128, nodes]); gates on ACT, combine on DVE.
  - The halo exchange is an AllGather of the updated g shard, split into
    4 chunk-collectives launched as soon as their node groups' GRU
    completes so they overlap the block's remaining gather/compute; the
    late-launching chunks are small so the exchange tail doesn't spill
    past the block boundary.  The table is double-buffered by block
    parity because the mid-block collectives would otherwise overwrite
    rows the current block is still gathering (a WAR hazard the Tile
    framework does not order for DRAM).  Weights are replicated.
"""

import os
import sys
from contextlib import ExitStack

sys.path.insert(0, "/opt/trn_rl_repo")

_DBG = set(os.environ.get("KDBG", "").split(",")) - {""}

import numpy as np
import ml_dtypes

BF16 = ml_dtypes.bfloat16

# ---------------------------------------------------------------- config

FULL_CFG = dict(
    N=100000,
    E=800000,
    F=128,          # embedding width
    IN=3,
    HIDDEN=32,
    BLOCKS=5,
    NCORES=8,
    NPC=12800,      # nodes per core (padded)
    GROUP=512,      # nodes per group (GRU matmul free dim)
    TILE=128,
    AGG=(8, 8, 7, 2),   # groups per AllGather chunk; every chunk <= 8 groups
                        # (32768 rows, int16-addressable windows), and the
                        # late-launching chunks are small so the exchange
                        # tail does not spill past the block boundary
    QMOD=0,         # 0 = all tiles use Q7 dma_gather; else Q7 iff t%QMOD==1
)


def _derived(cfg):
    d = dict(cfg)
    d["NPAD"] = d["NCORES"] * d["NPC"]
    d["NT"] = d["NPC"] // d["TILE"]          # tiles per core
    d["TPG"] = d["GROUP"] // d["TILE"]       # tiles per group
    d["NGROUPS"] = d["NPC"] // d["GROUP"]
    assert sum(d["AGG"]) == d["NGROUPS"]
    assert d["NPC"] % d["GROUP"] == 0 and d["GROUP"] % d["TILE"] == 0
    return d


# ---------------------------------------------------------------- host prep


def preprocess(cfg, inputs):
    c = _derived(cfg)
    N, E, F = c["N"], c["E"], c["F"]
    NC, NPC, NPAD = c["NCORES"], c["NPC"], c["NPAD"]
    TILE, GROUP, TPG, NT, NG = c["TILE"], c["GROUP"], c["TPG"], c["NT"], c["NGROUPS"]
    AGG = c["AGG"]

    edge_idx = np.asarray(inputs["edge_idx"])
    row = edge_idx[0].astype(np.int64)
    col = edge_idx[1].astype(np.int64)

    deg = (np.bincount(col, minlength=N).astype(np.float32) + 1.0)
    dinv = deg ** -0.5

    # ---- load-balancing permutation: snake over the 800 (core,tile) buckets
    NB = NC * NT
    indeg = np.bincount(col, minlength=N)
    order = np.argsort(-indeg, kind="stable")
    rank = np.empty(N, np.int64)
    rank[order] = np.arange(N)
    rnd = rank // NB                      # round (0..124), exactly 125 rounds
    bpos = rank % NB
    bucket = np.where(rnd % 2 == 0, bpos, NB - 1 - bpos)
    slot = rnd                            # slot within bucket (< 128)
    core_of_node = bucket // NT
    tile_of_node = bucket % NT
    pos_local = tile_of_node * TILE + slot            # position within core
    pos_global = core_of_node * NPC + pos_local

    # inverse map: position -> node (or -1 for padding)
    node_of_pos = np.full(NPAD, -1, np.int64)
    node_of_pos[pos_global] = np.arange(N)

    # ---- AllGather chunking (groups per chunk) and gfull row layout
    gstart = np.concatenate([[0], np.cumsum(AGG)])    # group boundaries
    R = [a * GROUP for a in AGG]                      # rows per chunk per core
    base = np.concatenate([[0], np.cumsum([NC * r for r in R])])
    grp_of_pl = np.arange(NPC) // GROUP
    k_of_grp = np.zeros(NG, np.int64)
    for k in range(len(AGG)):
        k_of_grp[gstart[k]:gstart[k + 1]] = k
    k_of_pl = k_of_grp[grp_of_pl]
    row_in_chunk = np.arange(NPC) - gstart[k_of_pl] * GROUP

    def gfull_row(core, pl):
        k = k_of_pl[pl]
        return base[k] + core * np.array(R)[k] + row_in_chunk[pl]

    # ---- per-edge metadata
    pcol = pos_global[col]
    pl_row = pos_local[row]
    src_gfull = (base[k_of_pl[pl_row]]
                 + core_of_node[row] * np.array(R)[k_of_pl[pl_row]]
                 + row_in_chunk[pl_row])

    c_d = pcol // NPC
    pl_col = pcol % NPC
    t_all = pl_col // TILE
    vloc = pl_col % TILE
    QMOD = c["QMOD"]
    NWIN = len(AGG)
    if QMOD == 0:
        is_q7_t = np.ones(NT, bool)                # all tiles on Q7 gather
    else:
        is_q7_t = (np.arange(NT) % QMOD) == 1      # per-tile mechanism
    is_q7 = is_q7_t[t_all]
    w_src = k_of_pl[pl_row]                        # source AG chunk / window
    w_eff = np.where(is_q7, w_src, -1)             # -1 = DGE cell

    # chunk counts per cell; cells per tile: DGE tile -> 1 cell, Q7 -> NWIN
    Md = np.zeros(NT, np.int64)                    # DGE chunks per tile
    Mq = np.zeros((NT, NWIN), np.int64)            # Q7 chunks per (tile, w)
    cnt_d = np.zeros((NC, NT), np.int64)
    cnt_q = np.zeros((NC, NT, NWIN), np.int64)
    dm = ~is_q7
    np.add.at(cnt_d, (c_d[dm], t_all[dm]), 1)
    qm = is_q7
    np.add.at(cnt_q, (c_d[qm], t_all[qm], w_src[qm]), 1)
    Md = -(-cnt_d.max(axis=0) // 128)
    Mq = -(-cnt_q.max(axis=0) // 128)
    Md[~is_q7_t] = np.maximum(Md[~is_q7_t], 1)     # every tile >= 1 chunk
    empty_q = is_q7_t & (Mq.sum(axis=1) == 0)
    Mq[empty_q, 0] = 1

    # per-group chunk sequence: [DGE tiles (tile-major)] then [w: Q7 tiles]
    # chunk position tables
    seq_tile = []       # per group: list of tile_in_group per chunk
    dge_pos = np.full((NT,), -1, np.int64)         # chunk pos of tile's DGE run
    q7_pos = np.full((NT, NWIN), -1, np.int64)     # chunk pos of (tile,w) run
    SOFF = [0] * (NG + 1)
    qcall = []          # per group: list of (w, pos, nch)
    ndge = []           # per group: number of DGE chunks
    for g in range(NG):
        seq = []
        for ti in range(TPG):
            t = g * TPG + ti
            if not is_q7_t[t]:
                dge_pos[t] = len(seq)
                seq += [ti] * int(Md[t])
        ndge.append(len(seq))
        calls = []
        for w in range(NWIN):
            w0 = len(seq)
            for ti in range(TPG):
                t = g * TPG + ti
                if is_q7_t[t] and Mq[t, w] > 0:
                    q7_pos[t, w] = len(seq)
                    seq += [ti] * int(Mq[t, w])
            if len(seq) > w0:
                calls.append((w, w0, len(seq) - w0))
        qcall.append(calls)
        seq_tile.append(seq)
        SOFF[g + 1] = SOFF[g] + len(seq)
    TOTCH = SOFF[NG]
    C_g = [SOFF[g + 1] - SOFF[g] for g in range(NG)]
    CMAXG = max(C_g)

    # chunk meta: (tile_in_group, is_first_of_tile, is_last_of_tile)
    chunk_meta = []
    for g in range(NG):
        seq = seq_tile[g]
        first = {}
        last = {}
        for i, ti in enumerate(seq):
            first.setdefault(ti, i)
            last[ti] = i
        chunk_meta.append(
            [(ti, i == first[ti], i == last[ti]) for i, ti in enumerate(seq)])

    # ---- per-core edge fill
    wkey = np.where(is_q7, w_src, -1)
    order2 = np.lexsort((src_gfull, wkey, t_all, c_d))
    ro_g = src_gfull[order2]
    co_v = vloc[order2]
    to_ = t_all[order2]
    cdo = c_d[order2]
    wo_ = wkey[order2]
    key = (cdo * NT + to_) * (NWIN + 1) + (wo_ + 1)
    runstart = np.zeros(len(key), bool)
    runstart[0] = True
    runstart[1:] = key[1:] != key[:-1]
    run_first = np.where(runstart)[0]
    run_id = np.cumsum(runstart) - 1
    erank = np.arange(len(key)) - run_first[run_id]
    go_ = to_ // TPG
    cell_pos = np.where(wo_ < 0, dge_pos[to_], q7_pos[to_, np.maximum(wo_, 0)])
    chunk = np.asarray(SOFF)[go_] + cell_pos + erank // 128   # global chunk id
    p = erank % 128
    sval = dinv[col[order2]].astype(BF16)

    # DGE idx layout: chunks 0..ndge[g]-1 of each group, compacted
    DOFF = [0] * (NG + 1)
    for g in range(NG):
        DOFF[g + 1] = DOFF[g] + ndge[g]
    TOTD = DOFF[NG]
    # Q7 idx16 stream layout: per (g, w) call, 128*nch values
    IOFF = {}
    acc16 = 0
    for g in range(NG):
        for (w, pos, nch) in qcall[g]:
            IOFF[(g, w)] = acc16
            acc16 += 128 * nch
    TOTQ = max(acc16, 16)

    base_w = [int(base[k]) for k in range(NWIN)]

    x = np.asarray(inputs["x"], np.float32)

    per_core = []
    for cc in range(NC):
        m = cdo == cc
        S_img = np.zeros((128, TOTCH, 128), BF16)
        S_img[p[m], chunk[m], co_v[m]] = sval[m]

        # DGE int32 idx (global gfull rows), compact chunk ids
        idx32 = np.zeros((128, max(TOTD, 1)), np.int32)
        md = m & (wo_ < 0)
        local_ch = chunk[md] - np.asarray(SOFF)[go_[md]]      # pos within group
        dge_ch = np.asarray(DOFF)[go_[md]] + local_ch         # compact id
        idx32[p[md], dge_ch] = ro_g[md]

        # Q7 int16 idx stream (window-relative), wrapped
        callpos = {}
        for g in range(NG):
            for (w, pos, nch) in qcall[g]:
                callpos[(g, w)] = pos
        idx_lin = np.zeros(TOTQ, np.int16)
        mq2 = m & (wo_ >= 0)
        if mq2.any():
            goq = go_[mq2]
            wq = wo_[mq2]
            ioff = np.array([IOFF[(gg, ww)] for gg, ww in zip(goq, wq)])
            cpos = np.array([callpos[(gg, ww)] for gg, ww in zip(goq, wq)])
            posq = chunk[mq2] - np.asarray(SOFF)[goq] - cpos
            flat = ioff + posq * 128 + p[mq2]
            idx_lin[flat] = (ro_g[mq2] - np.asarray(base_w)[wq]).astype(np.int16)
        idx16 = np.tile(
            np.ascontiguousarray(idx_lin.reshape(-1, 16).T), (8, 1))

        # node-side arrays in position order
        nid = node_of_pos[cc * NPC:(cc + 1) * NPC]    # [-1 for pads]
        valid = nid >= 0
        xT = np.zeros((c["IN"], NPC), np.float32)
        xT[:, valid] = x[nid[valid]].T
        dv = np.zeros(NPC, np.float32)
        dv[valid] = dinv[nid[valid]]
        dinv_sb = dv.reshape(NT, 128).T.copy()

        per_core.append(dict(
            idx=idx32,
            idx16=idx16,
            S=S_img,
            xT=xT,
            dinv=np.ascontiguousarray(dinv_sb),
        ))

    # ---- shared weights
    H = c["HIDDEN"]
    W_ih = np.asarray(inputs["W_ih"], np.float32)
    W_hh = np.asarray(inputs["W_hh"], np.float32)
    b_ih = np.asarray(inputs["b_ih"], np.float32)
    b_hh = np.asarray(inputs["b_hh"], np.float32)
    shared = dict(
        wemb=np.ascontiguousarray(np.asarray(inputs["W_embed"], np.float32).T),
        wih=np.ascontiguousarray(W_ih.T),
        whh=np.ascontiguousarray(W_hh.T),
        wdec=np.ascontiguousarray(np.asarray(inputs["W_dec"], np.float32).T),
        wout=np.ascontiguousarray(np.asarray(inputs["W_out"], np.float32).T),
        bemb=np.asarray(inputs["b_embed"], np.float32).reshape(F, 1),
        br=(b_ih[:F] + b_hh[:F]).reshape(F, 1),
        bzn=(-(b_ih[F:2 * F] + b_hh[F:2 * F])).reshape(F, 1),
        bin_=b_ih[2 * F:].reshape(F, 1),
        bhn=b_hh[2 * F:].reshape(1, F).astype(BF16),
        bdec=np.asarray(inputs["b_dec"], np.float32).reshape(H, 1),
        bout=np.asarray(inputs["b_out"], np.float32).reshape(1, 1),
        ones=np.ones((1, GROUP), BF16),
        ident=np.eye(128, dtype=np.float32),
    )
    for pc in per_core:
        pc.update(shared)

    meta = dict(cfg=c, chunk_meta=chunk_meta, SOFF=SOFF, C_g=C_g,
                TOTCH=TOTCH, CMAXG=CMAXG, R=R, base=base, gstart=gstart,
                node_of_pos=node_of_pos, pos_global=pos_global,
                ndge=ndge, DOFF=DOFF, TOTD=max(TOTD, 1), qcall=qcall,
                IOFF=IOFF, TOTQ=TOTQ, base_w=base_w)
    return per_core, meta


def postprocess(meta, results, core_ids):
    c = meta["cfg"]
    y_pos = np.concatenate(
        [np.asarray(results[i]["y"]).ravel() for i in core_ids])
    y = y_pos[meta["pos_global"]]
    return y.reshape(-1, 1).astype(np.float32)


# ---------------------------------------------------------------- builder


def build_kernel(meta, reps=1):
    import concourse.bacc as bacc
    import concourse.bass as bass
    import concourse.mybir as mybir
    import concourse.tile as tile

    c = meta["cfg"]
    F, IN, H = c["F"], c["IN"], c["HIDDEN"]
    NPC, NPAD, GROUP, TPG, NG = c["NPC"], c["NPAD"], c["GROUP"], c["TPG"], c["NGROUPS"]
    NT, BLOCKS, AGG = c["NT"], c["BLOCKS"], c["AGG"]
    TOTCH, CMAXG = meta["TOTCH"], meta["CMAXG"]
    chunk_meta, SOFF, C_g = meta["chunk_meta"], meta["SOFF"], meta["C_g"]
    R, base, gstart = meta["R"], meta["base"], meta["gstart"]
    ndge, DOFF, TOTD = meta["ndge"], meta["DOFF"], meta["TOTD"]
    qcall, IOFF, TOTQ = meta["qcall"], meta["IOFF"], meta["TOTQ"]
    base_w = meta["base_w"]
    NAG = len(AGG)
    rows_w = [int(base[k + 1] - base[k]) for k in range(NAG)]

    f32 = mybir.dt.float32
    f32r = mybir.dt.float32r
    bf16 = mybir.dt.bfloat16
    i32 = mybir.dt.int32
    i16 = mybir.dt.int16
    AF = mybir.ActivationFunctionType
    ALU = mybir.AluOpType

    nc = bacc.Bacc("TRN2", target_bir_lowering=False, debug=False,
                   num_devices=c["NCORES"])

    din = {}
    def dram_in(name, shape, dt):
        din[name] = nc.dram_tensor(name, shape, dt, kind="ExternalInput")
        return din[name]

    idx_d = dram_in("idx", [128, TOTD], i32)
    idx16_d = dram_in("idx16", [128, TOTQ // 16], i16)
    S_d = dram_in("S", [128, TOTCH, 128], bf16)
    xT_d = dram_in("xT", [IN, NPC], f32r)
    dinv_d = dram_in("dinv", [128, NT], f32)
    wemb_d = dram_in("wemb", [IN, F], f32r)
    wih_d = dram_in("wih", [F, 3 * F], f32r)
    whh_d = dram_in("whh", [F, 3 * F], f32r)
    wdec_d = dram_in("wdec", [F, H], f32r)
    wout_d = dram_in("wout", [H, 1], f32r)
    bemb_d = dram_in("bemb", [F, 1], f32)
    br_d = dram_in("br", [F, 1], f32)
    bzn_d = dram_in("bzn", [F, 1], f32)
    bin_d = dram_in("bin_", [F, 1], f32)
    bhn_d = dram_in("bhn", [1, F], bf16)
    bdec_d = dram_in("bdec", [H, 1], f32)
    bout_d = dram_in("bout", [1, 1], f32)
    ones_d = dram_in("ones", [1, GROUP], bf16)
    ident_d = dram_in("ident", [128, 128], f32)

    y_d = nc.dram_tensor("y", [NG, GROUP], f32, kind="ExternalOutput")

    # Double-buffered by block parity: the AllGather chunks of block b fire
    # mid-block and must not clobber the table block b is still gathering
    # from (Tile does not order that WAR hazard on DRAM).
    gshard = [[nc.dram_tensor(f"gshard{par}_{k}", [R[k], F], bf16)
               for k in range(NAG)] for par in range(2)]
    gfull = [nc.dram_tensor(f"gfull{par}", [NPAD, F], bf16,
                            addr_space="Shared") for par in range(2)]

    # ---- resident sbuf
    idx_sb = nc.alloc_sbuf_tensor("idx_sb", [128, TOTD], i32)
    idx16_sb = nc.alloc_sbuf_tensor("idx16_sb", [128, TOTQ // 16], i16)
    dinv_sb = nc.alloc_sbuf_tensor("dinv_sb", [128, NT], f32)
    wemb_sb = nc.alloc_sbuf_tensor("wemb_sb", [IN, F], f32r)
    wih_sb = nc.alloc_sbuf_tensor("wih_sb", [F, 3 * F], f32r)
    whh_sb = nc.alloc_sbuf_tensor("whh_sb", [F, 3 * F], f32r)
    wdec_sb = nc.alloc_sbuf_tensor("wdec_sb", [F, H], f32r)
    wout_sb = nc.alloc_sbuf_tensor("wout_sb", [H, 1], f32r)
    bemb_sb = nc.alloc_sbuf_tensor("bemb_sb", [F, 1], f32)
    br_sb = nc.alloc_sbuf_tensor("br_sb", [F, 1], f32)
    bzn_sb = nc.alloc_sbuf_tensor("bzn_sb", [F, 1], f32)
    bin_sb = nc.alloc_sbuf_tensor("bin_sb", [F, 1], f32)
    bhn_sb = nc.alloc_sbuf_tensor("bhn_sb", [1, F], bf16)
    bdec_sb = nc.alloc_sbuf_tensor("bdec_sb", [H, 1], f32)
    bout_sb = nc.alloc_sbuf_tensor("bout_sb", [1, 1], f32)
    ones_sb = nc.alloc_sbuf_tensor("ones_sb", [1, GROUP], bf16)
    ident_sb = nc.alloc_sbuf_tensor("ident_sb", [128, 128], f32)

    hf = [nc.alloc_sbuf_tensor(f"hf{g}", [F, GROUP], f32) for g in range(NG)]
    zt = [nc.alloc_sbuf_tensor(f"zt{g}", [F, GROUP], f32) for g in range(NG)]

    rg = [list(range(c["NCORES"]))]

    with tile.TileContext(nc) as tc:
        for sb, d in [(idx_sb, idx_d), (idx16_sb, idx16_d),
                      (dinv_sb, dinv_d), (wemb_sb, wemb_d),
                      (wih_sb, wih_d), (whh_sb, whh_d), (wdec_sb, wdec_d),
                      (wout_sb, wout_d), (bemb_sb, bemb_d), (br_sb, br_d),
                      (bzn_sb, bzn_d), (bin_sb, bin_d), (bhn_sb, bhn_d),
                      (bdec_sb, bdec_d), (bout_sb, bout_d), (ones_sb, ones_d),
                      (ident_sb, ident_d)]:
            nc.sync.dma_start(sb[...], d[...])

        pools = ExitStack()
        gpool = pools.enter_context(tc.tile_pool(name="gath", bufs=2))
        spool = pools.enter_context(tc.tile_pool(name="spool", bufs=2))
        xpool = pools.enter_context(tc.tile_pool(name="xpool", bufs=2))
        apool = pools.enter_context(tc.tile_pool(name="apool", bufs=2))
        tpool = pools.enter_context(tc.tile_pool(name="tpool", bufs=2))
        gopool = pools.enter_context(tc.tile_pool(name="gopool", bufs=2))
        ps2 = pools.enter_context(tc.tile_pool(name="ps2", bufs=2, space="PSUM"))
        ps1 = pools.enter_context(tc.tile_pool(name="ps1", bufs=1, space="PSUM"))
        ypool = pools.enter_context(tc.tile_pool(name="ypool", bufs=2))

        def produce_g(g, par):
            """hf[g] -> transpose -> dinv scale -> bf16 -> gshard rows."""
            k = int(np.searchsorted(gstart, g, side="right")) - 1
            ps_tr = ps2.tile([128, GROUP], f32, tag="tr")
            for t in range(TPG):
                nc.tensor.transpose(
                    ps_tr[:, t * 128:(t + 1) * 128],
                    hf[g][:, t * 128:(t + 1) * 128],
                    ident_sb[...],
                )
            gsb = gopool.tile([128, TPG, 128], bf16, tag="gout")
            for t in range(TPG):
                kk = g * TPG + t
                nc.vector.tensor_scalar(
                    gsb[:, t, :], ps_tr[:, t * 128:(t + 1) * 128],
                    dinv_sb[:, kk:kk + 1], None, op0=ALU.mult,
                )
            for t in range(TPG):
                r0 = (g - gstart[k]) * GROUP + t * 128
                nc.sync.dma_start(gshard[par][k][r0:r0 + 128, :], gsb[:, t, :])

        def allgather(k, par):
            if "noag" in _DBG:
                return
            nc.gpsimd.collective_compute(
                "AllGather", ALU.bypass, replica_groups=rg,
                ins=[gshard[par][k][...].opt()],
                outs=[gfull[par][int(base[k]):int(base[k + 1]), :].opt()],
            )

        def gathers(g, gath, par):
            """Hybrid gather for group g: DGE indirect DMA (one call per
            chunk; HW takes one index per partition) runs concurrently with
            Q7 SWDGE dma_gather calls (one per source window) on ring 1."""
            for ci in range(ndge[g]):
                nc.gpsimd.indirect_dma_start(
                    out=gath[:, ci, :],
                    out_offset=None,
                    in_=gfull[par][...],
                    in_offset=bass.IndirectOffsetOnAxis(
                        ap=idx_sb[:, DOFF[g] + ci:DOFF[g] + ci + 1], axis=0),
                )
            for (w, pos, nch) in qcall[g]:
                n_i = 128 * nch
                io = IOFF[(g, w)]
                nc.gpsimd.dma_gather(
                    gath[:, pos:pos + nch, :],
                    gfull[par][base_w[w]:base_w[w] + rows_w[w], :],
                    idx16_sb[:, io // 16:(io + n_i) // 16],
                    n_i, n_i, F, single_packet=False,
                )

        for _rep in range(reps):
            # ---- embed phase
            for g in range(NG):
                xg = xpool.tile([IN, GROUP], f32r, tag="xg")
                nc.sync.dma_start(xg[...], xT_d[:, g * GROUP:(g + 1) * GROUP])
                ps_h = ps1.tile([F, GROUP], f32, tag="psr")
                nc.tensor.matmul(ps_h[...], wemb_sb[...], xg[...], start=True, stop=True)
                nc.scalar.activation(hf[g][...], ps_h[...], AF.Relu, bias=bemb_sb[...])
                nc.scalar.activation(zt[g][...], hf[g][...], AF.Copy)
                produce_g(g, 0)
                if g + 1 in gstart[1:]:
                    allgather(int(np.searchsorted(gstart, g, side="right")) - 1, 0)

            # ---- message passing blocks
            for blk in range(BLOCKS):
                last = blk == BLOCKS - 1
                rpar = blk % 2          # table parity this block reads
                wpar = (blk + 1) % 2    # table parity this block produces
                for g in range(NG):
                    gath = gpool.tile([128, CMAXG, 128], bf16, tag="gath")
                    gathers(g, gath, rpar)
                    s_t = spool.tile([128, CMAXG, 128], bf16, tag="S")
                    nc.sync.dma_start(
                        s_t[:, :C_g[g], :],
                        S_d[:, SOFF[g]:SOFF[g] + C_g[g], :]
                    )
                    ps_aggr = ps2.tile([F, GROUP], f32, tag="aggr")
                    # emit grouped by tile so each tile's PSUM accumulation
                    # run is contiguous (chunk ids may interleave tiles)
                    for ci, (ti, st, sp) in sorted(
                            enumerate(chunk_meta[g]),
                            key=lambda x: (x[1][0], x[0])):
                        nc.tensor.matmul(
                            ps_aggr[:, ti * 128:(ti + 1) * 128],
                            gath[:, ci, :], s_t[:, ci, :],
                            start=st, stop=sp,
                        )
                    aggrF = apool.tile([F, GROUP], f32r, tag="aggrF")
                    nc.scalar.activation(aggrF[...], ps_aggr[...], AF.Copy)

                    hr = apool.tile([F, GROUP], f32r, tag="hr")
                    nc.scalar.activation(hr[...], hf[g][...], AF.Copy)
                    ps_r = ps1.tile([F, GROUP], f32, tag="psr")
                    nc.tensor.matmul(ps_r[...], wih_sb[:, 0:F], hr[...], start=True, stop=False)
                    nc.tensor.matmul(ps_r[...], whh_sb[:, 0:F], aggrF[...], start=False, stop=True)
                    ps_z = ps1.tile([F, GROUP], f32, tag="psz")
                    nc.tensor.matmul(ps_z[...], wih_sb[:, F:2 * F], hr[...], start=True, stop=False)
                    nc.tensor.matmul(ps_z[...], whh_sb[:, F:2 * F], aggrF[...], start=False, stop=True)
                    ps_gin = ps1.tile([F, GROUP], f32, tag="psgin")
                    nc.tensor.matmul(ps_gin[...], wih_sb[:, 2 * F:3 * F], hr[...], start=True, stop=True)
                    ps_ghn = ps1.tile([F, GROUP], f32, tag="psghn")
                    nc.tensor.matmul(ps_ghn[...], whh_sb[:, 2 * F:3 * F], aggrF[...], start=True, stop=False)
                    nc.tensor.matmul(ps_ghn[...], bhn_sb[...], ones_sb[...], start=False, stop=True)

                    r_sb = tpool.tile([F, GROUP], f32, tag="r")
                    nc.scalar.activation(r_sb[...], ps_r[...], AF.Sigmoid, bias=br_sb[...])
                    zc_sb = tpool.tile([F, GROUP], f32, tag="zc")
                    nc.scalar.activation(zc_sb[...], ps_z[...], AF.Sigmoid,
                                         bias=bzn_sb[...], scale=-1.0)
                    tA = tpool.tile([F, GROUP], f32, tag="tA")
                    nc.vector.tensor_tensor(tA[...], r_sb[...], ps_ghn[...], op=ALU.mult)
                    tB = tpool.tile([F, GROUP], f32, tag="tB")
                    nc.vector.tensor_tensor(tB[...], tA[...], ps_gin[...], op=ALU.add)
                    n_sb = tpool.tile([F, GROUP], f32, tag="n")
                    nc.scalar.activation(n_sb[...], tB[...], AF.Tanh, bias=bin_sb[...])
                    d_sb = tpool.tile([F, GROUP], f32, tag="tA")
                    nc.vector.tensor_tensor(d_sb[...], n_sb[...], aggrF[...], op=ALU.subtract)
                    e_sb = tpool.tile([F, GROUP], f32, tag="tB")
                    nc.vector.tensor_tensor(e_sb[...], zc_sb[...], d_sb[...], op=ALU.mult)
                    nc.vector.tensor_tensor(hf[g][...], aggrF[...], e_sb[...], op=ALU.add)
                    nc.vector.tensor_tensor(zt[g][...], zt[g][...], hf[g][...], op=ALU.max)
                    if not last:
                        produce_g(g, wpar)
                        if g + 1 in gstart[1:]:
                            allgather(int(np.searchsorted(gstart, g, side="right")) - 1, wpar)

            # ---- decoder
            for g in range(NG):
                zr = apool.tile([F, GROUP], f32r, tag="hr")
                nc.scalar.activation(zr[...], zt[g][...], AF.Copy)
                ps_dec = ps1.tile([H, GROUP], f32, tag="psr")
                nc.tensor.matmul(ps_dec[...], wdec_sb[...], zr[...], start=True, stop=True)
                decT = tpool.tile([H, GROUP], f32r, tag="dec")
                nc.scalar.activation(decT[...], ps_dec[...], AF.Relu, bias=bdec_sb[...])
                ps_y = ps1.tile([1, GROUP], f32, tag="psz")
                nc.tensor.matmul(ps_y[...], wout_sb[...], decT[...], start=True, stop=True)
                y_sb = ypool.tile([1, GROUP], f32, tag="y")
                nc.scalar.activation(y_sb[...], ps_y[...], AF.Copy)
                nc.vector.tensor_scalar(y_sb[...], y_sb[...], bout_sb[0:1, 0:1], None, op0=ALU.add)
                nc.sync.dma_start(y_d[g:g + 1, :], y_sb[0:1, :])

        pools.close()

    nc.compile()
    return nc


# ---------------------------------------------------------------- entry


def kernel(**inputs):
    from concourse.bass_utils import run_bass_kernel_spmd

    cfg = FULL_CFG
    c = _derived(cfg)
    per_core, meta = preprocess(cfg, inputs)
    nc = build_kernel(meta)
    core_ids = list(range(c["NCORES"]))
    res = run_bass_kernel_spmd(nc, per_core, core_ids).results
    return postprocess(meta, res, core_ids)
